# revision 36
# baseline (speedup 1.0000x reference)
"""Trainium2 Bass kernel for nn_Attention (Bahdanau-style attention pooling).

Computation (reference):
    cat    = concat([hidden broadcast over S, encoder_outputs], -1)   # [B,S,2048]
    energy = tanh(cat @ W_attn + b_attn)                              # [B,S,512]
    scores = energy @ w_v                                             # [B,S]
    att    = softmax(scores, axis=1)
    ctx    = att @ encoder_outputs                                    # [B,1024]

Strategy: data-parallel over batch across 8 cores (2 batches/core).

The energy GEMM (the FLOP bulk) runs in fp8-e4m3 with DoubleRow perf mode:
each matmul contracts 2 k-chunks (256 rows) in 512 cycles, 2x the bf16 rate.
enc ships twice: as e4m3 (x8 scale, k-chunk-paired layout feeding the PE)
and as bf16 (feeding the DVE context pass, which needs full precision).
W2 ships as e4m3 (x256 scale); the tanh activation descales via its scale
operand and adds the per-(h,b) hproj bias (hidden @ W1 + b, computed on-chip
in bf16 exactly like the baseline).

The fp8 quantization error of scores is cancelled to first order by a
host-precomputed rank-1 correction: linearizing score error through tanh
with per-(batch,h) mean tanh' (Gauss-Hermite over the pre-activation
distribution), both the W2-residual and enc-residual contributions collapse
to corr[b,s] = v_b . enc_q[s] + g_b . (enc - enc_q)[s], a tiny host GEMV
(~0.2% of kernel FLOPs). corr is shipped per-core and added into the score
PSUM group by one rank-1 matmul per j-block. Measured end-to-end rel err
~6e-3 (vs 2.2e-2 uncorrected).

Softmax skips the max subtraction (|scores| small, safe in fp32) and its
normalization happens on the host (kernel ships unnormalized context
columns plus per-block exp-sums).  The att row is broadcast to 128
partitions with a rank-1 PE matmul into PSUM, copied to SBUF as bf16 by the
scalar engine; context partials are fused multiply+reduce (STT accum) on
the vector engine over j-pair-wide (1024-col) bf16 tiles.
"""

import numpy as np
import ml_dtypes
from contextlib import ExitStack

import concourse.bass as bass
import concourse.tile as tile
from concourse import bacc, mybir
from concourse.bass_utils import run_bass_kernel_spmd

F32 = mybir.dt.float32
BF16 = mybir.dt.bfloat16
E4 = mybir.dt.float8e4

NCORES = 8
B = 16            # total batches
B2 = B // NCORES  # batches per core
S = 4096          # sequence length
D = 1024          # encoder feature dim (= 2H)
H = 512           # attention hidden dim
KT = D // 128     # d chunks (8)
PAIRS = KT // 2   # DoubleRow k-chunk pairs (4)
HC = H // 128     # h chunks (4)
SB = 512          # sequence block for energy/scores
NJ = S // SB      # 8 blocks
NQ = 4            # s quarters (DMA granularity; quarter == j-pair)
QW = S // NQ      # 1024
NJP = NJ // 2     # j-pairs per batch (== DMA quarters)

SE = 8.0          # enc fp8 scale
SW = 256.0        # W2 fp8 scale

AF = mybir.ActivationFunctionType
ALU = mybir.AluOpType
DR = mybir.MatmulPerfMode.DoubleRow

_cached_nc = None
_last_in_maps = None


def _build():
    nc = bacc.Bacc("TRN2", target_bir_lowering=False, debug=False)

    encH = nc.dram_tensor("encH", [B2, PAIRS, 128, 2, S], E4, kind="ExternalInput")
    encB = nc.dram_tensor("encB", [B2, KT, 128, S], BF16, kind="ExternalInput")
    hidT = nc.dram_tensor("hidT", [128, KT, 128], BF16, kind="ExternalInput")
    W1 = nc.dram_tensor("W1", [128, KT, H], BF16, kind="ExternalInput")
    W2 = nc.dram_tensor("W2", [128, PAIRS, 2, H], E4, kind="ExternalInput")
    bT = nc.dram_tensor("bT", [128, HC], F32, kind="ExternalInput")
    wvT = nc.dram_tensor("wvT", [128, HC], BF16, kind="ExternalInput")
    corrT = nc.dram_tensor("corrT", [B2, S], BF16, kind="ExternalInput")
    onesin = nc.dram_tensor("onesin", [1, 640], BF16, kind="ExternalInput")
    out = nc.dram_tensor("ctx_out", [B2, 128, KT], F32, kind="ExternalOutput")
    zout = nc.dram_tensor("z_out", [B2, NJP + 1], F32, kind="ExternalOutput")
    out_view = out.ap()

    with tile.TileContext(nc) as tc:
        with ExitStack() as ctx:
            const = ctx.enter_context(tc.tile_pool(name="const", bufs=1))
            onescb = const.tile([1, 640], BF16, name="onescb")
            nc.sync.dma_start(onescb, onesin.ap())
            onesc = onescb[0:1, :128]
            W2_sb = const.tile([128, PAIRS, 2, H], E4, name="W2_sb")
            nc.sync.dma_start(W2_sb, W2.ap())
            W1_sb = const.tile([128, KT, H], BF16, name="W1_sb")
            hid_sb = const.tile([128, KT, 128], BF16, name="hid_sb")
            nc.sync.dma_start(W1_sb, W1.ap())
            nc.sync.dma_start(hid_sb, hidT.ap())
            wv_sb = const.tile([128, HC], BF16, name="wv_sb")
            nc.sync.dma_start(wv_sb, wvT.ap())
            bT_sb = const.tile([128, HC], F32, name="bT_sb")
            nc.sync.dma_start(bT_sb, bT.ap())
            corr_sb = const.tile([1, B2 * S], BF16, name="corr_sb")
            for bb in range(B2):
                nc.sync.dma_start(corr_sb[0:1, bb * S:(bb + 1) * S],
                                  corrT.ap()[bb:bb + 1, :])
            hproj_sb = const.tile([128, HC * B2], F32, name="hproj_sb")

            # ---- main pools ----
            # enc fp8 pair tiles: fully resident, 8 x [128, 2, S] (64KB/part)
            hip = ctx.enter_context(tc.tile_pool(name="hip", bufs=2 * PAIRS))
            # enc bf16 quarter tiles, rotating with 3 quarters of lookahead
            encp = ctx.enter_context(tc.tile_pool(name="encp", bufs=3 * KT))
            ep = ctx.enter_context(tc.tile_pool(name="ep", bufs=6))
            arowp = ctx.enter_context(tc.tile_pool(name="arowp", bufs=3))
            arsp = ctx.enter_context(tc.tile_pool(name="arsp", bufs=5))
            zp = ctx.enter_context(tc.tile_pool(name="zp", bufs=2))
            scrp = ctx.enter_context(tc.tile_pool(name="scrp", bufs=4))
            trp = ctx.enter_context(tc.tile_pool(name="trp", bufs=2))
            partsp = ctx.enter_context(tc.tile_pool(name="partsp", bufs=18))
            ctxp = ctx.enter_context(tc.tile_pool(name="ctxp", bufs=2))
            # PSUM: pe 3 x 2-bank + ps 1 x 2-bank = 8 banks
            pe_pool = ctx.enter_context(
                tc.tile_pool(name="pe_pool", bufs=3, space="PSUM"))
            ps_pool = ctx.enter_context(
                tc.tile_pool(name="ps_pool", bufs=1, space="PSUM"))

            # PE warmup on the (tiny, first-to-land) ones tile: opens the HAM
            # clock gate while W2/enc stream in.
            wps = ps_pool.tile([128, QW], F32, name="wps", tag="ps")
            for _ in range(16):
                nc.tensor.matmul(wps[:, 0:SB], W2_sb[:, 0, 0, 0:128],
                                 W2_sb[:, 0, 0, 0:SB], start=True, stop=True)

            # hi (fp8, PE food) is prioritized over encB (bf16, DVE food)
            # at the batch boundary so the PE never starves; encB quarters
            # arrive just ahead of each j-pair's context pass.
            hi_tiles = {}
            for b in range(B2):
                for t in range(PAIRS):
                    hi_tiles[(b, t)] = hip.tile([128, 2, S], E4,
                                                name=f"hi_{b}_{t}", tag="hi")
            enc_tiles = {}
            DMA_ORDER = [("hi", 0, 0), ("encB", 0, 0), ("hi", 0, 1),
                         ("encB", 0, 1), ("hi", 0, 2), ("hi", 0, 3),
                         ("hi", 1, 0), ("hi", 1, 1), ("encB", 0, 2),
                         ("hi", 1, 2), ("hi", 1, 3), ("encB", 0, 3),
                         ("encB", 1, 0), ("encB", 1, 1), ("encB", 1, 2),
                         ("encB", 1, 3)]
            for kind, bb, q in DMA_ORDER:
                qs = slice(q * QW, (q + 1) * QW)
                if kind == "hi":
                    for t in range(PAIRS):
                        nc.sync.dma_start(hi_tiles[(bb, t)][:, :, qs],
                                          encH.ap()[bb, t, :, :, qs])
                else:
                    for k in range(KT):
                        et = encp.tile([128, QW], BF16,
                                       name=f"enc_{bb}_{k}_{q}", tag="enc")
                        nc.sync.dma_start(et, encB.ap()[bb, k, :, qs])
                        enc_tiles[(bb, k, q)] = et

            zrows, ctxts, partss = {}, {}, {}
            eTs = {}     # (b, jp, hc) -> energyT pair tile [128, 1024]
            pss = {}     # (b, jp) -> scores psum pair tile [1, 1024]
            arows = {}   # (b, jp) -> exp(scores) pair row tile [1, 1024]
            arps = {}    # (b, jp) -> att broadcast SBUF pair tile

            def is_fin(b, jp):
                return b == B2 - 1 and jp == NJP - 1

            pes = {}

            def emit_energy_mm(b, jp, hc):
                """8 DoubleRow fp8 matmuls accumulating pre-energy^T for
                a j-pair."""
                pe = pe_pool.tile([128, QW], F32, name=f"pe_{b}_{jp}_{hc}",
                                  tag="pe")
                for half in range(2):
                    c0 = jp * QW + half * SB
                    for t in range(PAIRS):
                        nc.tensor.matmul(
                            pe[:, half * SB:(half + 1) * SB],
                            W2_sb[:, t, :, hc * 128:(hc + 1) * 128],
                            hi_tiles[(b, t)][:, :, c0:c0 + SB],
                            start=(t == 0), stop=(t == PAIRS - 1),
                            perf_mode=DR,
                        )
                pes[(b, jp, hc)] = pe

            def emit_tanh(b, jp, hc):
                """1024-wide tanh (descale + per-(h,b) hproj bias)."""
                eT = ep.tile([128, QW], BF16, name=f"eT_{b}_{jp}_{hc}",
                             tag="eT")
                nc.scalar.activation(
                    eT, pes.pop((b, jp, hc)), AF.Tanh,
                    bias=hproj_sb[:, hc * B2 + b: hc * B2 + b + 1],
                    scale=1.0 / (SE * SW),
                )
                eTs[(b, jp, hc)] = eT

            def emit_energy(b, jp, hc):
                emit_energy_mm(b, jp, hc)
                emit_tanh(b, jp, hc)

            def emit_score(b, jp):
                """Whole score burst for a j-pair: the correction rows open
                both accumulation groups, then all 4 h-chunk matmuls per
                half; group lives a single step (1 PSUM pair slot)."""
                ps = ps_pool.tile([1, QW], F32, name=f"ps_{b}_{jp}", tag="ps")
                pss[(b, jp)] = ps
                for half in range(2):
                    c0 = b * S + jp * QW + half * SB
                    nc.tensor.matmul(
                        ps[0:1, half * SB:(half + 1) * SB],
                        onescb[0:1, 0:1],
                        corr_sb[0:1, c0:c0 + SB],
                        start=True, stop=False,
                    )
                for hc in range(HC):
                    eT = eTs.pop((b, jp, hc))
                    for half in range(2):
                        nc.tensor.matmul(
                            ps[0:1, half * SB:(half + 1) * SB],
                            wv_sb[:, hc:hc + 1],
                            eT[:, half * SB:(half + 1) * SB],
                            start=False, stop=(hc == HC - 1),
                        )

            def emit_exp(b, jp):
                ps = pss.pop((b, jp))
                zrow = zrows[b]
                arow = arowp.tile([1, QW], BF16, name=f"arow_{b}_{jp}",
                                  tag="arow")
                arows[(b, jp)] = arow
                if is_fin(b, jp):
                    for half, col in ((0, jp), (1, NJP)):
                        hs = slice(half * SB, (half + 1) * SB)
                        nc.scalar.activation(
                            arow[0:1, hs], ps[0:1, hs], AF.Exp,
                            accum_out=zrow[:, col:col + 1])
                else:
                    nc.scalar.activation(arow, ps, AF.Exp,
                                         accum_out=zrow[:, jp:jp + 1])

            def emit_arep(b, jp):
                # broadcast both att rows to 128 partitions via PE into a
                # 2-bank PSUM tile, then wide bf16 copy to SBUF
                arow = arows.pop((b, jp))
                arep = pe_pool.tile([128, QW], F32,
                                    name=f"arep_{b}_{jp}", tag="pe")
                for half in range(2):
                    hs = slice(half * SB, (half + 1) * SB)
                    nc.tensor.matmul(arep[:, hs], onesc, arow[0:1, hs],
                                     start=True, stop=True)
                arps[(b, jp)] = arsp.tile([128, QW], BF16,
                                          name=f"arps_{b}_{jp}", tag="arps")
                if is_fin(b, jp):
                    for half in range(2):
                        hs = slice(half * SB, (half + 1) * SB)
                        nc.scalar.activation(arps[(b, jp)][:, hs],
                                             arep[:, hs], AF.Copy)
                else:
                    nc.scalar.activation(arps[(b, jp)], arep, AF.Copy)

            def emit_ctx(b, jp):
                """Fused multiply+reduce of att against enc bf16 quarter
                tiles on the vector engine.  The final pair runs as two
                half-tiles so the first half starts before the last
                j-block's att lands (shrinks the DVE tail)."""
                arp = arps.pop((b, jp))
                parts = partss[b]
                pieces = [(slice(0, SB), jp), (slice(SB, QW), NJP)] \
                    if is_fin(b, jp) else [(slice(0, QW), jp)]
                act_ks = set()
                for k in range(KT):
                    et = enc_tiles.pop((b, k, jp))
                    for hs, col in pieces:
                        if k in act_ks:
                            # offload: multiply at 2x bf16 on DVE, reduce on
                            # the scalar engine (Copy + accumulator)
                            sc = scrp.tile([128, QW], BF16,
                                           name=f"scr_{b}_{jp}_{k}_{col}",
                                           tag="scr")
                            nc.vector.tensor_tensor(
                                out=sc[:, hs], in0=et[:, hs], in1=arp[:, hs],
                                op=ALU.mult)
                            tr = trp.tile([128, QW], BF16,
                                          name=f"tr_{b}_{jp}_{k}_{col}",
                                          tag="tr")
                            nc.scalar.activation(
                                tr[:, hs], sc[:, hs], AF.Copy,
                                accum_out=parts[k][:, col:col + 1])
                        else:
                            sc = scrp.tile([128, QW], BF16,
                                           name=f"scr_{b}_{jp}_{k}_{col}",
                                           tag="scr")
                            nc.vector.scalar_tensor_tensor(
                                out=sc[:, hs],
                                in0=et[:, hs],
                                scalar=1.0,
                                in1=arp[:, hs],
                                op0=ALU.mult,
                                op1=ALU.mult,
                                accum_out=parts[k][:, col:col + 1],
                            )

            def emit_tail(b):
                # normalization happens on host: ship zrow, raw ctx columns
                nc.sync.dma_start(zout.ap()[b:b + 1, :], zrows[b])
                ctxt = ctxts[b]
                ncols = NJP + 1 if b == B2 - 1 else NJP
                for k in range(KT):
                    nc.vector.tensor_reduce(ctxt[:, k:k + 1],
                                            partss[b][k][:, 0:ncols],
                                            axis=mybir.AxisListType.X,
                                            op=ALU.add)
                nc.sync.dma_start(out_view[b], ctxt)

            def start_batch(b):
                zrows[b] = zp.tile([1, NJP + 1], F32, name=f"zrow_{b}",
                                   tag="zrow")
                nc.vector.memset(zrows[b][:, NJP:NJP + 1], 0.0)
                ctxts[b] = ctxp.tile([128, KT], F32, name=f"ctx_{b}",
                                     tag="ctx")
                partss[b] = [partsp.tile([128, NJP + 1], F32,
                                         name=f"parts_{b}_{k}", tag="parts")
                             for k in range(KT)]

            # ramp: the first two steps' energy matmuls run before hproj so
            # the PE goes (and stays) dense right after warmup; their tanhs
            # follow the hproj computation.
            emit_energy_mm(0, 0, 0)
            emit_energy_mm(0, 0, 1)

            # hproj^T[h, b] = (hidden @ W1 + b_attn)^T; only the tanh bias
            # needs it, first used by pair 0's tanh
            for hcx in range(HC):
                ph = ps_pool.tile([128, QW], F32, name=f"ph_{hcx}", tag="ps")
                for k in range(KT):
                    nc.tensor.matmul(
                        ph[:, 0:B2],
                        W1_sb[:, k, hcx * 128:(hcx + 1) * 128],
                        hid_sb[:, k, 0:B2],
                        start=(k == 0), stop=(k == KT - 1),
                    )
                nc.vector.tensor_scalar_add(
                    hproj_sb[:, hcx * B2:(hcx + 1) * B2],
                    ph[:, 0:B2], bT_sb[:, hcx:hcx + 1])

            # ONE continuous software pipeline across both batches: scores
            # lag energy by 1 step, exp by 2, the arep broadcast by 3, the
            # DVE context pass by 4.  Batch b's drain overlaps batch b+1's
            # energy ramp, so the PE never idles at the boundary.
            steps = [(b, jp, hc) for b in range(B2) for jp in range(NJP)
                     for hc in range(HC)]
            for b in range(B2):
                start_batch(b)
            for i, (b, jp, hc) in enumerate(steps):
                if i <= 1:
                    emit_tanh(b, jp, hc)   # matmuls were emitted pre-hproj
                else:
                    emit_energy(b, jp, hc)
                if i >= 2 and steps[i - 2][2] == HC - 1:
                    emit_score(*steps[i - 2][:2])
                    emit_exp(*steps[i - 2][:2])
                if i >= 3 and steps[i - 3][2] == HC - 1:
                    emit_arep(*steps[i - 3][:2])
                if i >= 4 and steps[i - 4][2] == HC - 1:
                    emit_ctx(*steps[i - 4][:2])
                    if steps[i - 4][1] == NJP - 1:
                        emit_tail(steps[i - 4][0])
            emit_score(B2 - 1, NJP - 1)
            emit_exp(B2 - 1, NJP - 1)
            emit_arep(B2 - 1, NJP - 1)
            emit_ctx(B2 - 1, NJP - 1)
            emit_tail(B2 - 1)

    nc.compile()
    return nc


def _get_nc():
    global _cached_nc
    if _cached_nc is None:
        _cached_nc = _build()
    return _cached_nc


def _chunk_pk(a):
    """[1024, X] -> [128, 8, X] with element (p, k, x) = a[k*128+p, x]."""
    x = a.reshape(KT, 128, -1).transpose(1, 0, 2)
    return np.ascontiguousarray(x)


def kernel(hidden, encoder_outputs, W_attn, b_attn, w_v, **_kw):
    hidden = np.asarray(hidden, dtype=np.float32)
    encoder_outputs = np.asarray(encoder_outputs, dtype=np.float32)
    W_attn = np.asarray(W_attn, dtype=np.float32)
    b_attn = np.asarray(b_attn, dtype=np.float32)
    w_v = np.asarray(w_v, dtype=np.float32)
    E4n = ml_dtypes.float8_e4m3
    BFn = ml_dtypes.bfloat16

    W1f, W2f = W_attn[:D], W_attn[D:]

    # ---- host-side layout prep (sharding + tiling layout choices) ----
    encT = np.ascontiguousarray(encoder_outputs.transpose(0, 2, 1))  # [B,D,S]
    enc_hi8 = (encT * SE).astype(E4n)                                # [B,D,S]
    # paired fp8 layout for DoubleRow: [B, PAIRS, 128, 2, S]
    encH = np.ascontiguousarray(
        enc_hi8.reshape(B, PAIRS, 2, 128, S).transpose(0, 1, 3, 2, 4))
    encB = np.ascontiguousarray(
        encT.reshape(B, KT, 128, S)).astype(BFn)                     # [B,KT,128,S]

    hidTn = _chunk_pk(hidden.T)                     # [128, 8, 16]
    hidT = np.zeros((128, KT, 128), np.float32)
    hidT[:, :, :B] = hidTn
    hidT = hidT.astype(BFn)
    W1 = _chunk_pk(W1f).astype(BFn)                 # [128, KT, H]
    W2q8 = (W2f * SW).astype(E4n)                   # [1024, 512]
    W2 = np.ascontiguousarray(
        W2q8.reshape(PAIRS, 2, 128, H).transpose(2, 0, 1, 3))  # [128,4,2,H]
    bTv = np.ascontiguousarray(b_attn.reshape(HC, 128).T)      # [128, 4]
    wvT = np.ascontiguousarray(w_v.reshape(HC, 128).T).astype(BFn)

    # ---- rank-1 fp8-error score correction (see module docstring) ----
    enc_dev = enc_hi8.astype(np.float32) / SE                  # [B, D, S]
    denc = encT - enc_dev
    W2dev = W2q8.astype(np.float32) / SW                       # [1024, 512]
    dW2 = W2f - W2dev
    hproj = (hidden.astype(BFn).astype(np.float32)
             @ W1f.astype(BFn).astype(np.float32)) + b_attn    # [B, H]
    sig_h = np.linalg.norm(W2dev, axis=0)                      # [H]
    gh_x, gh_w = np.polynomial.hermite_e.hermegauss(41)
    rho = np.einsum(
        'bhk,k->bh',
        1.0 - np.tanh(hproj[:, :, None]
                      + sig_h[None, :, None] * gh_x[None, None, :]) ** 2,
        gh_w) / np.sqrt(2 * np.pi)                             # [B, H]
    vb = np.einsum('dh,h,bh->bd', dW2, w_v, rho)
    gb = np.einsum('dh,h,bh->bd', W2dev, w_v, rho)
    corr = (np.einsum('bd,bds->bs', vb, enc_dev)
            + np.einsum('bd,bds->bs', gb, denc)).astype(BFn)   # [B, S]

    in_maps = []
    for c in range(NCORES):
        sl = slice(c * B2, (c + 1) * B2)
        hidc = np.zeros((128, KT, 128), np.float32).astype(BFn)
        hidc[:, :, :B2] = hidT[:, :, c * B2:(c + 1) * B2]
        in_maps.append({
            "encH": np.ascontiguousarray(encH[sl]),
            "encB": np.ascontiguousarray(encB[sl]),
            "hidT": np.ascontiguousarray(hidc),
            "W1": W1,
            "W2": W2,
            "bT": bTv,
            "wvT": wvT,
            "corrT": np.ascontiguousarray(corr[sl]),
            "onesin": np.ones((1, 640), dtype=BFn),
        })

    global _last_in_maps
    _last_in_maps = in_maps
    nc = _get_nc()
    res = run_bass_kernel_spmd(nc, in_maps, core_ids=list(range(NCORES)))
    out = np.concatenate([res.results[c]["ctx_out"] for c in range(NCORES)],
                         axis=0)                    # [B, 128, KT]
    out = out.transpose(0, 2, 1).reshape(B, D)      # d = k*128 + p
    z = np.concatenate([res.results[c]["z_out"] for c in range(NCORES)],
                       axis=0).sum(axis=1, keepdims=True)
    return (out / z).astype(np.float32)
